# revision 6
# baseline (speedup 1.0000x reference)
"""Trainium2 Bass kernel for nn_Attention_53687091200195.

Reference computation (per batch b):
    Q = relu(x @ Wq + bq); K = relu(x @ Wk + bk); V = relu(x @ Wv + bv)
    S = Q @ K^T / sqrt(64); P = softmax(S, axis=-1); out = P @ V

Shapes: x [16, 2048, 64] f32, W* [64, 128] f32, b* [128] f32 -> out [16, 2048, 128].

Sharding: data-parallel over batch. 8 cores x 2 batches each; weights replicated.

Per-core design (SPMD, identical program):
  - Token-permuted layout: internal token index n~ = j*128 + p maps to real token
    p*16 + j.  Attention is permutation-equivariant over tokens, so computing on
    permuted tokens and writing output through the inverse permutation is exact,
    and it makes both the x-load and out-store DMAs fully contiguous per partition.
  - xT [c=64, n] built via PE transposes; QT/KT/VT = relu(W^T x^T + b) with the
    bias/relu fused into ACT reading PSUM (bias is per-partition in this layout).
  - V tiles [m,128 x d,128] from PE transposes of VT.
  - Attention sweep per 1024-token query chunk: for each key tile m (16):
      S^T tile = K_m @ Q^T  (PE, fp32r), E = exp(S^T/8) (ACT, PSUM->SBUF),
      outT += V_m^T @ E (PE accum), then den += ones^T @ E (PE accum).
    Softmax normalization happens after PV: out = outT * (1/den), since
    softmax(S) @ V == (exp(S) @ V) / rowsum(exp(S)).  No max-subtraction is
    needed: scores are ~0.4 +- 0.15, exp is far from overflow in fp32.
  - fp32r (bitcast of fp32) runs the PE at 1 cycle/row vs fp32's 4.
"""

import numpy as np

import concourse.bass as bass
import concourse.mybir as mybir
import concourse.tile as tile
from concourse import bacc
from concourse.bass_utils import run_bass_kernel_spmd
from concourse.masks import make_identity

N_CORES = 8
B_PER_CORE = 2
N_TOK = 2048
C_IN = 64
D = 128
P = 128                      # partitions / tile size
N_TILES = N_TOK // P         # 16 key/token tiles per batch
N_CHUNK = 1024               # query-chunk width for the attention sweep
N_CHUNKS = N_TOK // N_CHUNK  # 2
SCALE = 1.0 / 8.0            # 1/sqrt(64)

F32 = mybir.dt.float32
F32R = mybir.dt.float32r


def _r(ap):
    """View an fp32 AP as fp32r for full-rate PE matmuls."""
    return ap.bitcast(F32R)


def build_program():
    nc = bacc.Bacc("TRN2", target_bir_lowering=False, debug=False,
                   num_devices=N_CORES)

    x = nc.dram_tensor("x", [B_PER_CORE, N_TOK, C_IN], F32, kind="ExternalInput").ap()
    wq = nc.dram_tensor("Wq", [C_IN, D], F32, kind="ExternalInput").ap()
    bq = nc.dram_tensor("bq", [D], F32, kind="ExternalInput").ap()
    wk = nc.dram_tensor("Wk", [C_IN, D], F32, kind="ExternalInput").ap()
    bk = nc.dram_tensor("bk", [D], F32, kind="ExternalInput").ap()
    wv = nc.dram_tensor("Wv", [C_IN, D], F32, kind="ExternalInput").ap()
    bv = nc.dram_tensor("bv", [D], F32, kind="ExternalInput").ap()
    out = nc.dram_tensor("out", [B_PER_CORE, N_TOK, D], F32, kind="ExternalOutput").ap()

    with tile.TileContext(nc) as tc:
        kernel_body(tc, out, x, (wq, bq), (wk, bk), (wv, bv))

    nc.compile()
    return nc


def kernel_body(tc, out, x, qw, kw, vw):
    nc = tc.nc
    from contextlib import ExitStack
    ctx = ExitStack()
    with ctx:
        consts = ctx.enter_context(tc.tile_pool(name="consts", bufs=1))
        perb = ctx.enter_context(tc.tile_pool(name="perb", bufs=2))
        epool = ctx.enter_context(tc.tile_pool(name="epool", bufs=1))
        ep = ctx.enter_context(tc.tile_pool(name="ep", bufs=2))
        # PSUM: st 2x[128,1024]=4 banks (also hosts [128,<=512] tiles via tags),
        # acc 2 banks, den 2 banks = 8 banks total.
        pst = ctx.enter_context(tc.tile_pool(name="pst", bufs=2, space="PSUM"))
        pacc = ctx.enter_context(tc.tile_pool(name="pacc", bufs=1, space="PSUM"))
        pden = ctx.enter_context(tc.tile_pool(name="pden", bufs=1, space="PSUM"))

        # --- constants ---
        identity = consts.tile([P, P], F32)
        make_identity(nc, identity[:])
        ones_f = consts.tile([P, P], F32)
        nc.vector.memset(ones_f[:], 1.0)
        ones = consts.tile([P, P], F32R)
        nc.vector.tensor_copy(out=ones[:], in_=ones_f[:])

        w_sb = {}
        b_sb = {}
        for name, (w, b) in (("q", qw), ("k", kw), ("v", vw)):
            wf = consts.tile([C_IN, D], F32, name=f"wf_{name}", tag=f"wf_{name}")
            nc.sync.dma_start(out=wf[:], in_=w[:])
            w_sb[name] = consts.tile([C_IN, D], F32R, name=f"w_{name}", tag=f"w_{name}")
            nc.vector.tensor_copy(out=w_sb[name][:], in_=wf[:])
            b_sb[name] = consts.tile([D, 1], F32, name=f"b_{name}", tag=f"b_{name}")
            nc.sync.dma_start(out=b_sb[name][:], in_=b[:])

        for b in range(B_PER_CORE):
            # --- load x[b] in token-permuted layout ---
            # x_nat[p, j, c] = x[b, p*16 + j, c]; contiguous 4KB per partition.
            x_nat = perb.tile([P, N_TILES, C_IN], F32, tag="x_nat")
            nc.sync.dma_start(
                out=x_nat[:],
                in_=bass.AP(
                    tensor=x.tensor,
                    offset=b * N_TOK * C_IN,
                    ap=[[N_TILES * C_IN, P], [C_IN, N_TILES], [1, C_IN]],
                ),
            )

            # --- xT [c, n~]: PE-transpose each [128, 64] tile ---
            xT = perb.tile([C_IN, N_TOK], F32R, tag="xT")
            for j in range(N_TILES):
                xt_ps = pst.tile([C_IN, P], F32, tag="st")
                nc.tensor.transpose(xt_ps[:], x_nat[:, j, :], identity[:])
                nc.vector.tensor_copy(out=xT[:, j * P:(j + 1) * P], in_=xt_ps[:])

            # --- projections: QT/KT/VT [d, n~] = relu(W^T xT + b) ---
            proj = {}
            for name in ("q", "k", "v"):
                dt_t = F32 if name == "v" else F32R
                t = perb.tile([D, N_TOK], dt_t, name=f"{name}T_b{b}", tag=f"{name}T")
                proj[name] = t
                for s in range(N_TOK // 512):
                    ps = pst.tile([P, 512], F32, tag="st")
                    nc.tensor.matmul(ps[:], w_sb[name][:],
                                     xT[:, s * 512:(s + 1) * 512],
                                     start=True, stop=True)
                    nc.scalar.activation(
                        out=t[:, s * 512:(s + 1) * 512], in_=ps[:],
                        func=mybir.ActivationFunctionType.Relu,
                        bias=b_sb[name][:], scale=1.0)
            qT, kT, vT = proj["q"], proj["k"], proj["v"]

            # --- V tiles [m, d] via PE transposes of VT ---
            v_sb = perb.tile([P, N_TILES, D], F32R, tag="v_sb")
            for j in range(N_TILES):
                vt_ps = pst.tile([P, P], F32, tag="st")
                nc.tensor.transpose(vt_ps[:], vT[:, j * P:(j + 1) * P], identity[:])
                nc.vector.tensor_copy(out=v_sb[:, j, :], in_=vt_ps[:])

            # --- attention sweep ---
            for chunk in range(N_CHUNKS):
                n0 = chunk * N_CHUNK
                e_all = epool.tile([P, N_TILES, N_CHUNK], F32R, tag="e_all")
                acc = pacc.tile([P, N_CHUNK], F32, tag="acc")
                den = pden.tile([P, N_CHUNK], F32, tag="den")

                for m in range(N_TILES):
                    st = pst.tile([P, N_CHUNK], F32, tag="st")
                    for h in range(N_CHUNK // 512):
                        nc.tensor.matmul(
                            st[:, h * 512:(h + 1) * 512],
                            kT[:, m * P:(m + 1) * P],
                            qT[:, n0 + h * 512:n0 + (h + 1) * 512],
                            start=True, stop=True)
                    # E = exp(S^T / 8), PSUM -> SBUF
                    nc.scalar.activation(
                        out=e_all[:, m, :], in_=st[:],
                        func=mybir.ActivationFunctionType.Exp, scale=SCALE)
                    # outT += V_m^T @ E
                    for h in range(N_CHUNK // 512):
                        nc.tensor.matmul(
                            acc[:, h * 512:(h + 1) * 512],
                            v_sb[:, m, :],
                            e_all[:, m, h * 512:(h + 1) * 512],
                            start=(m == 0), stop=(m == N_TILES - 1))

                # den[*, n] = sum_m E[m, n] (replicated across partitions)
                for m in range(N_TILES):
                    for h in range(N_CHUNK // 512):
                        nc.tensor.matmul(
                            den[:, h * 512:(h + 1) * 512],
                            ones[:],
                            e_all[:, m, h * 512:(h + 1) * 512],
                            start=(m == 0), stop=(m == N_TILES - 1))

                # normalize then transpose to [n~, d] and store
                recip = ep.tile([P, N_CHUNK], F32, tag="recip")
                nc.vector.reciprocal(out=recip[:], in_=den[:])
                outn = ep.tile([P, N_CHUNK], F32, tag="outn")
                nc.vector.tensor_tensor(out=outn[:], in0=acc[:], in1=recip[:],
                                        op=mybir.AluOpType.mult)

                o_sb = ep.tile([P, N_CHUNK // P, D], F32, tag="o_sb")
                for jt in range(N_CHUNK // P):
                    tr_ps = pst.tile([P, P], F32, tag="st")
                    nc.tensor.transpose(tr_ps[:], outn[:, jt * P:(jt + 1) * P],
                                        identity[:])
                    nc.vector.tensor_copy(out=o_sb[:, jt, :], in_=tr_ps[:])

                # out[b, p*16 + chunk*8 + jt, d] = o_sb[p, jt, d]
                nc.sync.dma_start(
                    out=bass.AP(
                        tensor=out.tensor,
                        offset=(b * N_TOK + chunk * (N_CHUNK // P)) * D,
                        ap=[[N_TILES * D, P], [D, N_CHUNK // P], [1, D]],
                    ),
                    in_=o_sb[:],
                )


_NC_CACHE = None


def _get_program():
    global _NC_CACHE
    if _NC_CACHE is None:
        _NC_CACHE = build_program()
    return _NC_CACHE


def kernel(x, Wq, bq, Wk, bk, Wv, bv, _trace=False):
    x = np.ascontiguousarray(np.asarray(x, dtype=np.float32))
    full_b = x.shape[0]
    assert full_b == N_CORES * B_PER_CORE, x.shape
    nc = _get_program()
    common = {
        "Wq": np.ascontiguousarray(np.asarray(Wq, np.float32)),
        "bq": np.ascontiguousarray(np.asarray(bq, np.float32)),
        "Wk": np.ascontiguousarray(np.asarray(Wk, np.float32)),
        "bk": np.ascontiguousarray(np.asarray(bk, np.float32)),
        "Wv": np.ascontiguousarray(np.asarray(Wv, np.float32)),
        "bv": np.ascontiguousarray(np.asarray(bv, np.float32)),
    }
    in_maps = [
        {"x": x[c * B_PER_CORE:(c + 1) * B_PER_CORE], **common}
        for c in range(N_CORES)
    ]
    res = run_bass_kernel_spmd(nc, in_maps, list(range(N_CORES)), trace=_trace)
    outs = np.concatenate([res.results[c]["out"] for c in range(N_CORES)], axis=0)
    if _trace:
        kernel.last_exec_time_ns = res.exec_time_ns
    return outs


# revision 8
# speedup vs baseline: 1.1556x; 1.1556x over previous
"""Trainium2 Bass kernel for nn_Attention_53687091200195.

Reference computation (per batch b):
    Q = relu(x @ Wq + bq); K = relu(x @ Wk + bk); V = relu(x @ Wv + bv)
    S = Q @ K^T / sqrt(64); P = softmax(S, axis=-1); out = P @ V

Shapes: x [16, 2048, 64] f32, W* [64, 128] f32, b* [128] f32 -> out [16, 2048, 128].

Sharding: data-parallel over batch. 8 cores x 2 batches each; weights replicated.

Per-core design (SPMD, identical program):
  - Token-permuted layout: internal token index n~ = j*128 + p maps to real token
    p*16 + j.  Attention is permutation-equivariant over tokens, so computing on
    permuted tokens and writing output through the inverse permutation is exact,
    and it makes both the x-load and out-store DMAs fully contiguous per partition.
  - xT [c=64, n] built via PE transposes; QT/KT/VT = relu(W^T x^T + b) with the
    bias/relu fused into ACT reading PSUM (bias is per-partition in this layout).
  - V tiles [m,128 x d,128] from PE transposes of VT.
  - Attention sweep per 1024-token query chunk: for each key tile m (16):
      S^T tile = K_m @ Q^T  (PE, fp32r), E = exp(S^T/8) (ACT, PSUM->SBUF),
      outT += V_m^T @ E (PE accum), then den += ones^T @ E (PE accum).
    Softmax normalization happens after PV: out = outT * (1/den), since
    softmax(S) @ V == (exp(S) @ V) / rowsum(exp(S)).  No max-subtraction is
    needed: scores are ~0.4 +- 0.15, exp is far from overflow in fp32.
  - fp32r (bitcast of fp32) runs the PE at 1 cycle/row vs fp32's 4.
"""

import numpy as np

import concourse.bass as bass
import concourse.mybir as mybir
import concourse.tile as tile
from concourse import bacc
from concourse.bass_utils import run_bass_kernel_spmd
from concourse.masks import make_identity

N_CORES = 8
B_PER_CORE = 2
N_TOK = 2048
C_IN = 64
D = 128
P = 128                      # partitions / tile size
N_TILES = N_TOK // P         # 16 key/token tiles per batch
N_CHUNK = 1024               # query-chunk width for the attention sweep
N_CHUNKS = N_TOK // N_CHUNK  # 2
SCALE = 1.0 / 8.0            # 1/sqrt(64)

F32 = mybir.dt.float32
F32R = mybir.dt.float32r


def _r(ap):
    """View an fp32 AP as fp32r for full-rate PE matmuls."""
    return ap.bitcast(F32R)


def build_program():
    nc = bacc.Bacc("TRN2", target_bir_lowering=False, debug=False,
                   num_devices=N_CORES)

    x = nc.dram_tensor("x", [B_PER_CORE, N_TOK, C_IN], F32, kind="ExternalInput").ap()
    wq = nc.dram_tensor("Wq", [C_IN, D], F32, kind="ExternalInput").ap()
    bq = nc.dram_tensor("bq", [D], F32, kind="ExternalInput").ap()
    wk = nc.dram_tensor("Wk", [C_IN, D], F32, kind="ExternalInput").ap()
    bk = nc.dram_tensor("bk", [D], F32, kind="ExternalInput").ap()
    wv = nc.dram_tensor("Wv", [C_IN, D], F32, kind="ExternalInput").ap()
    bv = nc.dram_tensor("bv", [D], F32, kind="ExternalInput").ap()
    out = nc.dram_tensor("out", [B_PER_CORE, N_TOK, D], F32, kind="ExternalOutput").ap()

    with tile.TileContext(nc) as tc:
        kernel_body(tc, out, x, (wq, bq), (wk, bk), (wv, bv))

    nc.compile()
    return nc


def kernel_body(tc, out, x, qw, kw, vw):
    nc = tc.nc
    from contextlib import ExitStack
    ctx = ExitStack()
    with ctx:
        consts = ctx.enter_context(tc.tile_pool(name="consts", bufs=1))
        perb = ctx.enter_context(tc.tile_pool(name="perb", bufs=2))
        epool = ctx.enter_context(tc.tile_pool(name="epool", bufs=1))
        ep = ctx.enter_context(tc.tile_pool(name="ep", bufs=2))
        # PSUM: st 2x[128,1024]=4 banks (also hosts [128,<=512] tiles via tags),
        # acc 2 banks, den 2 banks = 8 banks total.
        pst = ctx.enter_context(tc.tile_pool(name="pst", bufs=2, space="PSUM"))
        pacc = ctx.enter_context(tc.tile_pool(name="pacc", bufs=1, space="PSUM"))
        pden = ctx.enter_context(tc.tile_pool(name="pden", bufs=1, space="PSUM"))

        # --- constants ---
        identity = consts.tile([P, P], F32)
        make_identity(nc, identity[:])
        ones_f = consts.tile([P, 1], F32)
        nc.vector.memset(ones_f[:], 1.0)
        ones = consts.tile([P, 1], F32R)
        nc.vector.tensor_copy(out=ones[:], in_=ones_f[:])

        w_sb = {}
        b_sb = {}
        for name, (w, b) in (("q", qw), ("k", kw), ("v", vw)):
            wf = consts.tile([C_IN, D], F32, name=f"wf_{name}", tag=f"wf_{name}")
            nc.sync.dma_start(out=wf[:], in_=w[:])
            w_sb[name] = consts.tile([C_IN, D], F32R, name=f"w_{name}", tag=f"w_{name}")
            nc.vector.tensor_copy(out=w_sb[name][:], in_=wf[:])
            b_sb[name] = consts.tile([D, 1], F32, name=f"b_{name}", tag=f"b_{name}")
            nc.sync.dma_start(out=b_sb[name][:], in_=b[:])

        for b in range(B_PER_CORE):
            # --- load x[b] in token-permuted layout ---
            # x_nat[p, j, c] = x[b, p*16 + j, c]; contiguous 4KB per partition.
            x_nat = perb.tile([P, N_TILES, C_IN], F32, tag="x_nat")
            nc.sync.dma_start(
                out=x_nat[:],
                in_=bass.AP(
                    tensor=x.tensor,
                    offset=b * N_TOK * C_IN,
                    ap=[[N_TILES * C_IN, P], [C_IN, N_TILES], [1, C_IN]],
                ),
            )

            # --- xT [c, n~]: PE-transpose each [128, 64] tile ---
            xT = perb.tile([C_IN, N_TOK], F32R, tag="xT")
            for j in range(N_TILES):
                xt_ps = pst.tile([C_IN, P], F32, tag="st")
                nc.tensor.transpose(xt_ps[:], x_nat[:, j, :], identity[:])
                nc.vector.tensor_copy(out=xT[:, j * P:(j + 1) * P], in_=xt_ps[:])

            # --- projections: QT/KT/VT [d, n~] = relu(W^T xT + b) ---
            proj = {}
            for name in ("q", "k", "v"):
                dt_t = F32 if name == "v" else F32R
                t = perb.tile([D, N_TOK], dt_t, name=f"{name}T_b{b}", tag=f"{name}T")
                proj[name] = t
                for s in range(N_TOK // 512):
                    ps = pst.tile([P, 512], F32, tag="st")
                    nc.tensor.matmul(ps[:], w_sb[name][:],
                                     xT[:, s * 512:(s + 1) * 512],
                                     start=True, stop=True)
                    nc.vector.tensor_scalar(
                        out=t[:, s * 512:(s + 1) * 512], in0=ps[:],
                        scalar1=b_sb[name][:], scalar2=0.0,
                        op0=mybir.AluOpType.add, op1=mybir.AluOpType.max)
            qT, kT, vT = proj["q"], proj["k"], proj["v"]

            # --- V tiles [m, d] via PE transposes of VT ---
            v_sb = perb.tile([P, N_TILES, D], F32R, tag="v_sb")
            for j in range(N_TILES):
                vt_ps = pst.tile([P, P], F32, tag="st")
                nc.tensor.transpose(vt_ps[:], vT[:, j * P:(j + 1) * P], identity[:])
                nc.vector.tensor_copy(out=v_sb[:, j, :], in_=vt_ps[:])

            # --- attention sweep ---
            for chunk in range(N_CHUNKS):
                n0 = chunk * N_CHUNK
                e_all = epool.tile([P, N_TILES, N_CHUNK], F32R, tag="e_all")
                acc = pacc.tile([P, N_CHUNK], F32, tag="acc")
                den = pden.tile([1, N_CHUNK], F32, tag="den")

                for m in range(N_TILES):
                    st = pst.tile([P, N_CHUNK], F32, tag="st")
                    for h in range(N_CHUNK // 512):
                        nc.tensor.matmul(
                            st[:, h * 512:(h + 1) * 512],
                            kT[:, m * P:(m + 1) * P],
                            qT[:, n0 + h * 512:n0 + (h + 1) * 512],
                            start=True, stop=True)
                    # E = exp(S^T / 8), PSUM -> SBUF
                    nc.scalar.activation(
                        out=e_all[:, m, :], in_=st[:],
                        func=mybir.ActivationFunctionType.Exp, scale=SCALE)
                    # outT += V_m^T @ E
                    for h in range(N_CHUNK // 512):
                        nc.tensor.matmul(
                            acc[:, h * 512:(h + 1) * 512],
                            v_sb[:, m, :],
                            e_all[:, m, h * 512:(h + 1) * 512],
                            start=(m == 0), stop=(m == N_TILES - 1))

                # den[*, n] = sum_m E[m, n] (replicated across partitions)
                for m in range(N_TILES):
                    for h in range(N_CHUNK // 512):
                        nc.tensor.matmul(
                            den[:, h * 512:(h + 1) * 512],
                            ones[:],
                            e_all[:, m, h * 512:(h + 1) * 512],
                            start=(m == 0), stop=(m == N_TILES - 1))

                # normalize via per-query reciprocal after transposing.
                # den rows -> [n,1] via tiny PE transposes; out = outT^T * (1/den)
                den_sb = ep.tile([1, N_CHUNK], F32, tag="den_sb")
                nc.scalar.copy(out=den_sb[:], in_=den[:])
                outu = ep.tile([P, N_CHUNK], F32, tag="outu")
                nc.vector.tensor_copy(out=outu[:], in_=acc[:])

                den_t = pst.tile([P, N_CHUNK // P], F32, tag="st")
                for jt in range(N_CHUNK // P):
                    nc.tensor.transpose(den_t[:, jt:jt + 1],
                                        den_sb[:, jt * P:(jt + 1) * P],
                                        identity[:1, :1])
                recip = ep.tile([P, N_CHUNK // P], F32, tag="recip")
                nc.vector.reciprocal(out=recip[:], in_=den_t[:])

                o_sb = ep.tile([P, N_CHUNK // P, D], F32, tag="o_sb")
                for jt in range(N_CHUNK // P):
                    tr_ps = pst.tile([P, P], F32, tag="st")
                    nc.tensor.transpose(tr_ps[:], outu[:, jt * P:(jt + 1) * P],
                                        identity[:])
                    nc.vector.tensor_scalar(
                        out=o_sb[:, jt, :], in0=tr_ps[:],
                        scalar1=recip[:, jt:jt + 1], scalar2=None,
                        op0=mybir.AluOpType.mult)

                # out[b, p*16 + chunk*8 + jt, d] = o_sb[p, jt, d]
                nc.sync.dma_start(
                    out=bass.AP(
                        tensor=out.tensor,
                        offset=(b * N_TOK + chunk * (N_CHUNK // P)) * D,
                        ap=[[N_TILES * D, P], [D, N_CHUNK // P], [1, D]],
                    ),
                    in_=o_sb[:],
                )


_NC_CACHE = None


def _get_program():
    global _NC_CACHE
    if _NC_CACHE is None:
        _NC_CACHE = build_program()
    return _NC_CACHE


def kernel(x, Wq, bq, Wk, bk, Wv, bv, _trace=False):
    x = np.ascontiguousarray(np.asarray(x, dtype=np.float32))
    full_b = x.shape[0]
    assert full_b == N_CORES * B_PER_CORE, x.shape
    nc = _get_program()
    common = {
        "Wq": np.ascontiguousarray(np.asarray(Wq, np.float32)),
        "bq": np.ascontiguousarray(np.asarray(bq, np.float32)),
        "Wk": np.ascontiguousarray(np.asarray(Wk, np.float32)),
        "bk": np.ascontiguousarray(np.asarray(bk, np.float32)),
        "Wv": np.ascontiguousarray(np.asarray(Wv, np.float32)),
        "bv": np.ascontiguousarray(np.asarray(bv, np.float32)),
    }
    in_maps = [
        {"x": x[c * B_PER_CORE:(c + 1) * B_PER_CORE], **common}
        for c in range(N_CORES)
    ]
    res = run_bass_kernel_spmd(nc, in_maps, list(range(N_CORES)), trace=_trace)
    outs = np.concatenate([res.results[c]["out"] for c in range(N_CORES)], axis=0)
    if _trace:
        kernel.last_exec_time_ns = res.exec_time_ns
    return outs


# revision 10
# speedup vs baseline: 1.2232x; 1.0585x over previous
"""Trainium2 Bass kernel for nn_Attention_53687091200195.

Reference computation (per batch b):
    Q = relu(x @ Wq + bq); K = relu(x @ Wk + bk); V = relu(x @ Wv + bv)
    S = Q @ K^T / sqrt(64); P = softmax(S, axis=-1); out = P @ V

Shapes: x [16, 2048, 64] f32, W* [64, 128] f32, b* [128] f32 -> out [16, 2048, 128].

Sharding: data-parallel over batch. 8 cores x 2 batches each; weights replicated.

Per-core design (SPMD, identical program):
  - Token-permuted layout: internal token index n~ = j*128 + p maps to real token
    p*16 + j.  Attention is permutation-equivariant over tokens, so computing on
    permuted tokens and writing output through the inverse permutation is exact,
    and it makes both the x-load and out-store DMAs fully contiguous per partition.
  - xT [c=64, n] built via PE transposes; QT/KT/VT = relu(W^T x^T + b) with the
    bias/relu fused into ACT reading PSUM (bias is per-partition in this layout).
  - V tiles [m,128 x d,128] from PE transposes of VT.
  - Attention sweep per 1024-token query chunk: for each key tile m (16):
      S^T tile = K_m @ Q^T  (PE, fp32r), E = exp(S^T/8) (ACT, PSUM->SBUF),
      outT += V_m^T @ E (PE accum), then den += ones^T @ E (PE accum).
    Softmax normalization happens after PV: out = outT * (1/den), since
    softmax(S) @ V == (exp(S) @ V) / rowsum(exp(S)).  No max-subtraction is
    needed: scores are ~0.4 +- 0.15, exp is far from overflow in fp32.
  - fp32r (bitcast of fp32) runs the PE at 1 cycle/row vs fp32's 4.
"""

import numpy as np

import concourse.bass as bass
import concourse.mybir as mybir
import concourse.tile as tile
from concourse import bacc
from concourse.bass_utils import run_bass_kernel_spmd
from concourse.masks import make_identity

N_CORES = 8
B_PER_CORE = 2
N_TOK = 2048
C_IN = 64
D = 128
P = 128                      # partitions / tile size
N_TILES = N_TOK // P         # 16 key/token tiles per batch
N_CHUNK = 1024               # query-chunk width for the attention sweep
N_CHUNKS = N_TOK // N_CHUNK  # 2
SCALE = 1.0 / 8.0            # 1/sqrt(64)

F32 = mybir.dt.float32
F32R = mybir.dt.float32r


def _r(ap):
    """View an fp32 AP as fp32r for full-rate PE matmuls."""
    return ap.bitcast(F32R)


def build_program():
    nc = bacc.Bacc("TRN2", target_bir_lowering=False, debug=False,
                   num_devices=N_CORES)

    x = nc.dram_tensor("x", [B_PER_CORE, N_TOK, C_IN], F32, kind="ExternalInput").ap()
    wq = nc.dram_tensor("Wq", [C_IN, D], F32, kind="ExternalInput").ap()
    bq = nc.dram_tensor("bq", [D], F32, kind="ExternalInput").ap()
    wk = nc.dram_tensor("Wk", [C_IN, D], F32, kind="ExternalInput").ap()
    bk = nc.dram_tensor("bk", [D], F32, kind="ExternalInput").ap()
    wv = nc.dram_tensor("Wv", [C_IN, D], F32, kind="ExternalInput").ap()
    bv = nc.dram_tensor("bv", [D], F32, kind="ExternalInput").ap()
    out = nc.dram_tensor("out", [B_PER_CORE, N_TOK, D], F32, kind="ExternalOutput").ap()

    with tile.TileContext(nc) as tc:
        kernel_body(tc, out, x, (wq, bq), (wk, bk), (wv, bv))

    nc.compile()
    return nc


def kernel_body(tc, out, x, qw, kw, vw):
    nc = tc.nc
    from contextlib import ExitStack
    ctx = ExitStack()
    with ctx:
        consts = ctx.enter_context(tc.tile_pool(name="consts", bufs=1))
        perb = ctx.enter_context(tc.tile_pool(name="perb", bufs=2))
        epool = ctx.enter_context(tc.tile_pool(name="epool", bufs=1))
        ep = ctx.enter_context(tc.tile_pool(name="ep", bufs=2))
        # PSUM: st 2x[128,1024]=4 banks (also hosts [128,<=512] tiles via tags),
        # acc 2 banks, den 2 banks = 8 banks total.
        pst = ctx.enter_context(tc.tile_pool(name="pst", bufs=2, space="PSUM"))
        pacc = ctx.enter_context(tc.tile_pool(name="pacc", bufs=1, space="PSUM"))
        pden = ctx.enter_context(tc.tile_pool(name="pden", bufs=1, space="PSUM"))

        # --- constants ---
        identity = consts.tile([P, P], F32)
        make_identity(nc, identity[:])
        identity_r = consts.tile([P, P], F32R)
        nc.vector.tensor_copy(out=identity_r[:], in_=identity[:])
        ones_f = consts.tile([P, 1], F32)
        nc.vector.memset(ones_f[:], 1.0)
        ones = consts.tile([P, 1], F32R)
        nc.vector.tensor_copy(out=ones[:], in_=ones_f[:])

        w_sb = {}
        b_sb = {}
        for name, (w, b) in (("q", qw), ("k", kw), ("v", vw)):
            wf = consts.tile([C_IN, D], F32, name=f"wf_{name}", tag=f"wf_{name}")
            nc.sync.dma_start(out=wf[:], in_=w[:])
            w_sb[name] = consts.tile([C_IN, D], F32R, name=f"w_{name}", tag=f"w_{name}")
            nc.vector.tensor_copy(out=w_sb[name][:], in_=wf[:])
            b_sb[name] = consts.tile([D, 1], F32, name=f"b_{name}", tag=f"b_{name}")
            nc.sync.dma_start(out=b_sb[name][:], in_=b[:])

        for b in range(B_PER_CORE):
            # --- load x[b] in token-permuted layout ---
            # x_nat[p, j, c] = x[b, p*16 + j, c]; contiguous 4KB per partition.
            x_nat = perb.tile([P, N_TILES, C_IN], F32, tag="x_nat")
            nc.sync.dma_start(
                out=x_nat[:],
                in_=bass.AP(
                    tensor=x.tensor,
                    offset=b * N_TOK * C_IN,
                    ap=[[N_TILES * C_IN, P], [C_IN, N_TILES], [1, C_IN]],
                ),
            )

            # --- xT [c, n~]: PE-transpose each [128, 64] tile ---
            xT = perb.tile([C_IN, N_TOK], F32R, tag="xT")
            for j in range(N_TILES):
                xt_ps = pst.tile([C_IN, P], F32, tag="st")
                nc.tensor.transpose(xt_ps[:], x_nat[:, j, :], identity[:])
                nc.vector.tensor_copy(out=xT[:, j * P:(j + 1) * P], in_=xt_ps[:])

            # --- projections: QT/KT/VT [d, n~] = relu(W^T xT + b) ---
            proj = {}
            for name in ("q", "k", "v"):
                t = perb.tile([D, N_TOK], F32R, name=f"{name}T_b{b}", tag=f"{name}T")
                proj[name] = t
                for s in range(N_TOK // 512):
                    ps = pst.tile([P, 512], F32, tag="st")
                    nc.tensor.matmul(ps[:], w_sb[name][:],
                                     xT[:, s * 512:(s + 1) * 512],
                                     start=True, stop=True)
                    nc.vector.tensor_scalar(
                        out=t[:, s * 512:(s + 1) * 512], in0=ps[:],
                        scalar1=b_sb[name][:], scalar2=0.0,
                        op0=mybir.AluOpType.add, op1=mybir.AluOpType.max)
            qT, kT, vT = proj["q"], proj["k"], proj["v"]

            # --- V tiles [m, d] via PE transposes of VT ---
            v_sb = perb.tile([P, N_TILES, D], F32R, tag="v_sb")
            for j in range(N_TILES):
                vt_ps = pst.tile([P, P], F32R, tag="st")
                nc.tensor.transpose(vt_ps[:], vT[:, j * P:(j + 1) * P], identity_r[:])
                nc.vector.tensor_copy(out=v_sb[:, j, :], in_=vt_ps[:])

            # --- attention sweep ---
            for chunk in range(N_CHUNKS):
                n0 = chunk * N_CHUNK
                e_all = epool.tile([P, N_TILES, N_CHUNK], F32R, tag="e_all")
                acc = pacc.tile([P, N_CHUNK], F32, tag="acc")
                den = pden.tile([1, N_CHUNK], F32, tag="den")

                for m in range(N_TILES):
                    st = pst.tile([P, N_CHUNK], F32, tag="st")
                    for h in range(N_CHUNK // 512):
                        nc.tensor.matmul(
                            st[:, h * 512:(h + 1) * 512],
                            kT[:, m * P:(m + 1) * P],
                            qT[:, n0 + h * 512:n0 + (h + 1) * 512],
                            start=True, stop=True)
                    # E = exp(S^T / 8), PSUM -> SBUF
                    nc.scalar.activation(
                        out=e_all[:, m, :], in_=st[:],
                        func=mybir.ActivationFunctionType.Exp, scale=SCALE)
                    # outT += V_m^T @ E
                    for h in range(N_CHUNK // 512):
                        nc.tensor.matmul(
                            acc[:, h * 512:(h + 1) * 512],
                            v_sb[:, m, :],
                            e_all[:, m, h * 512:(h + 1) * 512],
                            start=(m == 0), stop=(m == N_TILES - 1))

                # Copy unnormalized outT to SBUF (DVE) as soon as PV is done,
                # then interleave the out-transposes with the den matmuls so the
                # PE stream stays dense (HAM stays warm).
                outu = ep.tile([P, N_CHUNK], F32, tag="outu")
                nc.vector.tensor_copy(out=outu[:], in_=acc[:])

                o_sb = ep.tile([P, N_CHUNK // P, D], F32, tag="o_sb")
                for m in range(N_TILES):
                    for h in range(N_CHUNK // 512):
                        nc.tensor.matmul(
                            den[:, h * 512:(h + 1) * 512],
                            ones[:],
                            e_all[:, m, h * 512:(h + 1) * 512],
                            start=(m == 0), stop=(m == N_TILES - 1))
                    if m % 2 == 1 and m // 2 < N_CHUNK // P:
                        jt = m // 2
                        tr_ps = pst.tile([P, P], F32, tag="st",
                                         name=f"tr_{b}_{chunk}_{jt}")
                        nc.tensor.transpose(tr_ps[:],
                                            outu[:, jt * P:(jt + 1) * P],
                                            identity[:])
                        nc.vector.tensor_copy(out=o_sb[:, jt, :], in_=tr_ps[:])

                den_sb = ep.tile([1, N_CHUNK], F32, tag="den_sb")
                nc.scalar.copy(out=den_sb[:], in_=den[:])
                den_t = pst.tile([P, N_CHUNK // P], F32, tag="st")
                for jt in range(N_CHUNK // P):
                    nc.tensor.transpose(den_t[:, jt:jt + 1],
                                        den_sb[:, jt * P:(jt + 1) * P],
                                        identity[:1, :1])
                recip = ep.tile([P, N_CHUNK // P], F32, tag="recip")
                nc.vector.reciprocal(out=recip[:], in_=den_t[:])

                for jt in range(N_CHUNK // P):
                    nc.vector.tensor_scalar(
                        out=o_sb[:, jt, :], in0=o_sb[:, jt, :],
                        scalar1=recip[:, jt:jt + 1], scalar2=None,
                        op0=mybir.AluOpType.mult)

                # out[b, p*16 + chunk*8 + jt, d] = o_sb[p, jt, d]
                nc.sync.dma_start(
                    out=bass.AP(
                        tensor=out.tensor,
                        offset=(b * N_TOK + chunk * (N_CHUNK // P)) * D,
                        ap=[[N_TILES * D, P], [D, N_CHUNK // P], [1, D]],
                    ),
                    in_=o_sb[:],
                )


_NC_CACHE = None


def _get_program():
    global _NC_CACHE
    if _NC_CACHE is None:
        _NC_CACHE = build_program()
    return _NC_CACHE


def kernel(x, Wq, bq, Wk, bk, Wv, bv, _trace=False):
    x = np.ascontiguousarray(np.asarray(x, dtype=np.float32))
    full_b = x.shape[0]
    assert full_b == N_CORES * B_PER_CORE, x.shape
    nc = _get_program()
    common = {
        "Wq": np.ascontiguousarray(np.asarray(Wq, np.float32)),
        "bq": np.ascontiguousarray(np.asarray(bq, np.float32)),
        "Wk": np.ascontiguousarray(np.asarray(Wk, np.float32)),
        "bk": np.ascontiguousarray(np.asarray(bk, np.float32)),
        "Wv": np.ascontiguousarray(np.asarray(Wv, np.float32)),
        "bv": np.ascontiguousarray(np.asarray(bv, np.float32)),
    }
    in_maps = [
        {"x": x[c * B_PER_CORE:(c + 1) * B_PER_CORE], **common}
        for c in range(N_CORES)
    ]
    res = run_bass_kernel_spmd(nc, in_maps, list(range(N_CORES)), trace=_trace)
    outs = np.concatenate([res.results[c]["out"] for c in range(N_CORES)], axis=0)
    if _trace:
        kernel.last_exec_time_ns = res.exec_time_ns
    return outs
